# revision 2
# baseline (speedup 1.0000x reference)
"""NetVLAD Trainium2 kernel — data-parallel over N across 8 cores.

Per core: 4 images [C=128, P=4096].  Pipeline per 1024-pixel chunk:
  PE:   logits[p,k] = x_tile.T @ conv_wT   (x_tile stationary, shared with
        the x-transpose matmul x_tile.T @ I -> xT[p,c]); ssq[p] via
        xsq_tile.T @ ones.
  DVE/POOL/ACT: softmax over k in [pixel-partition, k-free] layout with
        per-pixel scalars held as [128, 8] stat columns and broadcast via
        step-0 access patterns.
  PE:   vlad^[k,c] += a_r.T-style accumulation: lhsT=a_r[:, :56],
        rhs=xT -> psum[56,128]; cluster mass s_k via rhs=n-col.
Final per image: vlad = term1 - s*cen, intra-normalize over k (via PE
transpose), global normalize, write [56,128] rows.
"""

import os
import sys

for _p in ("/opt/trn_rl_repo",):
    if _p not in sys.path:
        sys.path.insert(0, _p)

import numpy as np

NIMG = 4      # images per core
C = 128
K = 64
KE = 56
P = 4096
TPC = 8       # pixel tiles (128 px) per chunk
CH = TPC * 128
NCH = P // CH  # 4 chunks per image

_cache = {}


def _build():
    import concourse.bass as bass
    import concourse.mybir as mybir
    from concourse import bacc, tile

    f32 = mybir.dt.float32
    Alu = mybir.AluOpType
    Act = mybir.ActivationFunctionType

    nc = bacc.Bacc()
    x_in = nc.declare_dram_parameter("x", [NIMG, C, P], f32, isOutput=False)
    # packed consts: wT[0:64] | b8[64:576] | ident[576:704] | ones[704:832]
    # | cen[832:960] (partitions 0:56)
    cst_in = nc.declare_dram_parameter("consts", [C, 960], f32, isOutput=False)
    out_ext = nc.declare_dram_parameter("out", [NIMG, KE, C], f32, isOutput=True)
    dbg_ext = nc.declare_dram_parameter("dbg", [C, 680], f32, isOutput=True)

    with tile.TileContext(nc) as tc:
        with (
            tc.tile_pool(name="const", bufs=1) as cpool,
            tc.tile_pool(name="xin", bufs=3) as xpool,
            tc.tile_pool(name="work", bufs=2) as wpool,
            tc.tile_pool(name="stats", bufs=2) as spool,
            tc.tile_pool(name="fin", bufs=2) as fpool,
            tc.tile_pool(name="psL", bufs=2, space="PSUM") as pL,
            tc.tile_pool(name="psT", bufs=1, space="PSUM") as pT,
            tc.tile_pool(name="psS", bufs=2, space="PSUM") as pS,
            tc.tile_pool(name="psV", bufs=1, space="PSUM") as pV,
            tc.tile_pool(name="psF", bufs=1, space="PSUM") as pF,
        ):
            cst = cpool.tile([C, 960], f32, tag="cst")
            nc.gpsimd.dma_start(cst[:], cst_in[:])
            wT = cst[:, 0:K]
            b8 = cst[:, 64:64 + TPC * K]
            ident = cst[:, 576:576 + C]
            onesc = cst[:, 704:705]
            onesr = cst[0:1, 704:704 + C]
            cen = cst[0:KE, 832:832 + C]

            # PE warm-up: make PE observe the const-DMA semaphore once, so
            # later matmuls need at most one additional wait each.
            warm = pL.tile([C, TPC * K], f32, tag="L")
            nc.tensor.matmul(warm[0:1, 0:1], onesc, onesc,
                             start=True, stop=True)

            for img in range(NIMG):
                # [0:56, 0:128] vlad accum; [0:56, 128:129] s_k accum (via
                # the appended n-column in the rhs). Sole writer of its bank:
                # any other start=True matmul into this bank would clear it.
                psV = pV.tile([C, 160], f32, tag="psV")
                for ch in range(NCH):
                    xin = xpool.tile([C, CH], f32, tag="x")
                    nc.gpsimd.dma_start(xin[:], x_in[img, :, ch * CH:(ch + 1) * CH])
                    xsq = wpool.tile([C, CH], f32, tag="xsq")
                    nc.vector.tensor_mul(xsq[:], xin[:], xin[:])

                    psumL = pL.tile([C, TPC * K], f32, tag="L")
                    psumT = pT.tile([C, CH], f32, tag="T")
                    psumS = pS.tile([C, TPC], f32, tag="S")
                    for j in range(TPC):
                        xt = xin[:, j * 128:(j + 1) * 128]
                        nc.tensor.matmul(psumL[:, j * K:(j + 1) * K], xt, wT,
                                         start=True, stop=True)
                        nc.tensor.matmul(psumT[:, j * 128:(j + 1) * 128], xt,
                                         ident, start=True, stop=True)
                        nc.tensor.matmul(psumS[:, j:j + 1],
                                         xsq[:, j * 128:(j + 1) * 128], onesc,
                                         start=True, stop=True)

                    ncol = spool.tile([C, TPC], f32, tag="ncol")
                    nc.scalar.activation(ncol[:], psumS[:], Act.Sqrt)
                    invc = spool.tile([C, TPC], f32, tag="invc")
                    nc.vector.reciprocal(invc[:], ncol[:])

                    l3 = lambda t: t[:].rearrange("p (t k) -> p t k", k=K)
                    # u = raw * inv_n  (per-pixel scale, bcast along k)
                    lu = wpool.tile([C, TPC * K], f32, tag="lu")
                    nc.vector.tensor_tensor(
                        l3(lu), l3(psumL),
                        invc[:].broadcast_to([C, TPC, K]), Alu.mult)
                    # l = u + b   (bias per-k, pre-tiled 8x from host)
                    ll = wpool.tile([C, TPC * K], f32, tag="ll")
                    nc.vector.tensor_tensor(ll[:], lu[:], b8, Alu.add)
                    # m = max_k l
                    mcol = spool.tile([C, TPC], f32, tag="mcol")
                    nc.vector.tensor_reduce(mcol[:], l3(ll),
                                            axis=mybir.AxisListType.X,
                                            op=Alu.max)
                    # d = l - m
                    dd = wpool.tile([C, TPC * K], f32, tag="dd")
                    nc.vector.tensor_tensor(
                        l3(dd), l3(ll),
                        mcol[:].broadcast_to([C, TPC, K]), Alu.subtract)
                    # e = exp(d)
                    ee = wpool.tile([C, TPC * K], f32, tag="ee")
                    nc.scalar.activation(ee[:], dd[:], Act.Exp)
                    # sumexp
                    scol = spool.tile([C, TPC], f32, tag="scol")
                    nc.vector.tensor_reduce(scol[:], l3(ee),
                                            axis=mybir.AxisListType.X,
                                            op=Alu.add)
                    gcol = spool.tile([C, TPC], f32, tag="gcol")
                    nc.vector.reciprocal(gcol[:], scol[:])
                    rcol = spool.tile([C, TPC], f32, tag="rcol")
                    nc.vector.tensor_tensor(rcol[:], invc[:], gcol[:], Alu.mult)
                    # a_r = e * (inv_n / sumexp)
                    aa = wpool.tile([C, TPC * K], f32, tag="aa")
                    nc.vector.tensor_tensor(
                        l3(aa), l3(ee),
                        rcol[:].broadcast_to([C, TPC, K]), Alu.mult)
                    # xT evict into [x-tile | n-col] interleaved layout so
                    # each vlad rhs is one contiguous [128, 129] slab
                    xTs = wpool.tile([C, TPC * 129], f32, tag="xTs")
                    xTs_v = xTs[:].rearrange("p (t q) -> p t q", q=129)
                    nc.scalar.activation(
                        xTs_v[:, :, 0:128],
                        psumT[:].rearrange("p (t q) -> p t q", q=128),
                        Act.Copy)
                    nc.vector.tensor_copy(
                        xTs_v[:, :, 128:129],
                        ncol[:].broadcast_to([C, TPC, 1]))

                    if img == 0 and ch == 0:
                        nc.gpsimd.dma_start(dbg_ext[:, 0:TPC * K], aa[:])
                        nc.gpsimd.dma_start(dbg_ext[:, 512:512 + TPC], ncol[:])
                        nc.gpsimd.dma_start(dbg_ext[:, 520:520 + TPC], invc[:])
                        nc.gpsimd.dma_start(dbg_ext[:, 528:528 + TPC], mcol[:])
                        nc.gpsimd.dma_start(dbg_ext[:, 536:536 + TPC], scol[:])

                    # PE observer of the ACT semaphore (xTs write), so each
                    # vlad matmul below carries at most one (DVE) wait.
                    nc.tensor.matmul(psumT[0:1, 0:1], xTs[:, 0:1], onesc,
                                     start=True, stop=True)

                    first = ch == 0
                    last = ch == NCH - 1
                    for j in range(TPC):
                        nc.tensor.matmul(psV[0:KE, 0:129],
                                         aa[:, j * K:j * K + KE],
                                         xTs[:, j * 129:(j + 1) * 129],
                                         start=(first and j == 0),
                                         stop=(last and j == TPC - 1))

                # ---- per-image tail ----
                ps = pF.tile([C, 192], f32, tag="psF")
                negs = spool.tile([KE, 1], f32, tag="negs")
                nc.vector.tensor_scalar_mul(negs[:], psV[0:KE, 128:129], -1.0)
                vk = fpool.tile([KE, C], f32, tag="vk")
                nc.vector.scalar_tensor_tensor(vk[:], cen, negs[:],
                                               psV[0:KE, 0:C],
                                               Alu.mult, Alu.add)
                if img == 0:
                    nc.gpsimd.dma_start(
                        dbg_ext[0:KE, 544:544 + C], vk[:])
                    nc.gpsimd.dma_start(
                        dbg_ext[0:KE, 672:673], negs[:])
                # transpose -> [c, k]
                nc.tensor.matmul(ps[:, 0:KE], vk[:], ident[0:KE, 0:KE],
                                 start=True, stop=True)
                trash = fpool.tile([C, KE], f32, tag="trash")
                ssqk = spool.tile([C, 1], f32, tag="ssqk")
                nc.scalar.activation(trash[:], ps[:, 0:KE], Act.Square,
                                     accum_out=ssqk[:])
                nk = spool.tile([C, 1], f32, tag="nk")
                nc.scalar.activation(nk[:], ssqk[:], Act.Sqrt)
                nkc = spool.tile([C, 1], f32, tag="nkc")
                nc.vector.tensor_scalar_max(nkc[:], nk[:], 1e-12)
                invk = spool.tile([C, 1], f32, tag="invk")
                nc.vector.reciprocal(invk[:], nkc[:])
                t2 = spool.tile([C, 1], f32, tag="t2")
                nc.vector.scalar_tensor_tensor(t2[:], ssqk[:], invk[:], invk[:],
                                               Alu.mult, Alu.mult)
                # scalar matmuls go to a separate bank (start=True clears the
                # whole target bank, and ps[:, 0:KE] is still live)
                tiny = pL.tile([C, TPC * K], f32, tag="L")
                nc.tensor.matmul(tiny[0:1, 0:1], t2[:], onesc,
                                 start=True, stop=True)
                tot = spool.tile([1, 1], f32, tag="tot")
                nc.scalar.activation(tot[:], tiny[0:1, 0:1], Act.Sqrt)
                totc = spool.tile([1, 1], f32, tag="totc")
                nc.vector.tensor_scalar_max(totc[:], tot[:], 1e-12)
                fv = spool.tile([1, 1], f32, tag="fv")
                nc.vector.reciprocal(fv[:], totc[:])
                # broadcast fv to [128,1] via PE (wipes the tiny bank again;
                # tot was already evicted to SBUF)
                nc.tensor.matmul(tiny[:, 2:3], onesr, fv[:],
                                 start=True, stop=True)
                comb = spool.tile([C, 1], f32, tag="comb")
                nc.vector.tensor_tensor(comb[:], invk[:], tiny[:, 2:3], Alu.mult)
                vnT = fpool.tile([C, KE], f32, tag="vnT")
                nc.vector.tensor_scalar(vnT[:], ps[:, 0:KE], comb[:], None,
                                        Alu.mult)
                # transpose back -> [k, c]
                nc.tensor.matmul(ps[0:KE, 64:64 + C], vnT[:], ident,
                                 start=True, stop=True)
                ob = fpool.tile([KE, C], f32, tag="ob")
                nc.scalar.activation(ob[:], ps[0:KE, 64:64 + C], Act.Copy)
                nc.gpsimd.dma_start(out_ext[img], ob[:])

    nc.compile()
    return nc


def _get_nc():
    if "nc" not in _cache:
        _cache["nc"] = _build()
    return _cache["nc"]


def _make_in_maps(x, conv_w, conv_b, centroids):
    x = np.asarray(x, dtype=np.float32)
    conv_w = np.asarray(conv_w, dtype=np.float32)
    conv_b = np.asarray(conv_b, dtype=np.float32)
    centroids = np.asarray(centroids, dtype=np.float32)

    N = x.shape[0]
    n_cores = 8
    per = N // n_cores
    assert per == NIMG

    xr = x.reshape(N, C, P)
    cst = np.zeros((C, 960), dtype=np.float32)
    cst[:, 0:K] = conv_w.T
    cst[:, 64:64 + TPC * K] = np.tile(conv_b, TPC)[None, :]
    cst[:, 576:576 + C] = np.eye(C, dtype=np.float32)
    cst[:, 704:832] = 1.0
    cst[0:KE, 832:832 + C] = centroids[:KE]

    in_maps = []
    for i in range(n_cores):
        in_maps.append({
            "x": np.ascontiguousarray(xr[i * per:(i + 1) * per]),
            "consts": cst,
        })
    return in_maps


def kernel(x, conv_w, conv_b, centroids):
    from concourse.bass_utils import run_bass_kernel_spmd

    in_maps = _make_in_maps(x, conv_w, conv_b, centroids)
    n_cores = 8
    per = NIMG

    nc = _get_nc()
    res = run_bass_kernel_spmd(nc, in_maps, list(range(n_cores)))
    outs = [np.asarray(r["out"]).reshape(per, KE * C) for r in res.results]
    return np.concatenate(outs, axis=0)


if __name__ == "__main__":
    rng = np.random.default_rng(0)
    x = rng.standard_normal((32, C, 64, 64), dtype=np.float32)
    w = rng.standard_normal((K, C), dtype=np.float32)
    b = rng.standard_normal((K,), dtype=np.float32)
    c = rng.random((K, C), dtype=np.float32)
    out = kernel(x=x, conv_w=w, conv_b=b, centroids=c)
    print(out.shape, out.dtype)

